# revision 1
# baseline (speedup 1.0000x reference)
"""BitLinear (RMSNorm + int8-absmax activation quant + ternary weight quant + matmul)
on 8 Trainium2 NeuronCores.

Strategy:
  - Shard rows of x across cores (256 rows each): RMSNorm + local absmax.
  - Shard weight columns across cores ([4096, 512] each): local sum(|W|).
  - AllGather the two scalars -> global a_scale / b_scale (exact semantics).
  - Quantize activations to bf16 ints in [-127, 127] (exact in bf16),
    AllGather the quantized activation matrix (bf16, 16.8 MB total).
  - Quantize local weight shard to ternary bf16.
  - Matmul A_q @ B_t per core: lhsT tiles come from hardware DMA-transpose
    loads of the gathered bf16 activations; accumulate K=4096 in PSUM over
    32 k-tiles; dequant fused into the PSUM->SBUF copy.
  - Each core writes its [2048, 512] output column shard; host concatenates.

Self-contained: only needs numpy + the platform's concourse/bass libraries.
"""

import os
import sys

import numpy as np

for _p in ("/opt/trn_rl_repo", "/root/.axon_site/_ro/trn_rl_repo"):
    if os.path.isdir(_p) and _p not in sys.path:
        sys.path.append(_p)

import concourse.bass as bass
import concourse.tile as tile
from concourse import mybir
from concourse.bass_utils import run_bass_kernel_spmd

R = 8  # cores
M, K, N = 2048, 4096, 4096
M_LOC = M // R  # 256 rows of x per core
N_LOC = N // R  # 512 weight columns per core
P = 128
KT = K // P  # 32 k-tiles
MT_LOC = M_LOC // P  # 2 m-tiles per core
EPS_RMS = 1e-6
Q_CLIP = 1e-5
MAGIC = 12582912.0  # 1.5 * 2**23: (v + MAGIC) - MAGIC == round-to-nearest-even(v)
F32 = mybir.dt.float32
BF16 = mybir.dt.bfloat16
AX = mybir.AxisListType
ALU = mybir.AluOpType


def _split_waits(nc, max_waits=1):
    """This toolchain rejects instructions with several semaphore waits
    ("Too many sync wait commands"). Hoist excess waits onto no-op
    instructions just before the offender on the same engine."""
    counter = 0
    for f in nc.m.functions:
        for blk in f.blocks:
            new_insts = []
            for inst in blk.instructions:
                si = getattr(inst, "sync_info", None)
                waits = list(si.on_wait) if si is not None and si.on_wait else []
                if len(waits) > max_waits:
                    excess = waits[: len(waits) - max_waits]
                    keep = waits[len(waits) - max_waits :]
                    for i in range(0, len(excess), max_waits):
                        counter += 1
                        nop = mybir.InstNoOp(
                            name=f"waitsplit_{counter}_{inst.name}", ins=[], outs=[]
                        )
                        nop.engine = inst.engine
                        nop.bass_nofuse = True
                        nop.sync_info = mybir.SyncInfo(
                            on_wait=list(excess[i : i + max_waits]), on_update=[]
                        )
                        new_insts.append(nop)
                    si.on_wait = keep
                    inst.sync_info = si
                new_insts.append(inst)
            blk.instructions[:] = new_insts


def _bcast_ap(ap, p):
    """Broadcast a 1-D DRAM AP across p partitions (step-0 partition axis)."""
    return bass.AP(tensor=ap.tensor, offset=ap.offset, ap=[[0, p]] + list(ap.ap))


def build_kernel(reps=1, stop_after=None, mode=None):
    nc = bass.Bass(num_devices=R)
    rg = [list(range(R))]

    x_in = nc.declare_dram_parameter("x_loc", [M_LOC, K], F32, isOutput=False)
    w_in = nc.declare_dram_parameter("w_loc", [K, N_LOC], F32, isOutput=False)
    rms_in = nc.declare_dram_parameter("rms_w", [K], F32, isOutput=False)
    out_ext = nc.declare_dram_parameter("out_loc", [M, N_LOC], F32, isOutput=True)

    stats_loc = nc.dram_tensor("stats_loc", [P * 2], F32)
    stats_all = nc.dram_tensor("stats_all", [R * P * 2], F32, addr_space="Shared")
    wstat_loc = nc.dram_tensor("wstat_loc", [P], F32)
    wstat_all = nc.dram_tensor("wstat_all", [R * P], F32, addr_space="Shared")
    scal_dram = nc.dram_tensor("scal_dram", [1], F32)
    scbd_dram = nc.dram_tensor("scbd_dram", [2], F32)
    aq_loc = nc.dram_tensor("aq_loc", [M_LOC, K], BF16)
    aq_all_a = nc.dram_tensor("aq_all_a", [M // 2, K], BF16, addr_space="Shared")
    aq_all_b = nc.dram_tensor("aq_all_b", [M // 2, K], BF16, addr_space="Shared")

    with tile.TileContext(nc) as tc:
        ctxs = [
            tc.tile_pool(name="wres", bufs=1),
            tc.tile_pool(name="btres", bufs=1),
            tc.tile_pool(name="rmsp", bufs=1),
            tc.tile_pool(name="xz", bufs=2),
            tc.tile_pool(name="aq", bufs=2),
            tc.tile_pool(name="st", bufs=2),
            tc.tile_pool(name="lhs", bufs=12),
            tc.tile_pool(name="psum", bufs=8, space="PSUM"),
            tc.tile_pool(name="outp", bufs=4),
            tc.tile_pool(name="small", bufs=1),
        ]
        from contextlib import ExitStack

        with ExitStack() as es:
            (wres_p, bt_p, rms_p, xz_p, aq_p, st_p, lhs_p, psum_p, out_p, small_p) = [
                es.enter_context(c) for c in ctxs
            ]

            eps_t = small_p.tile([P, 1], F32)
            nc.vector.memset(eps_t, EPS_RMS)
            rms_b = rms_p.tile([P, K], F32)
            nc.scalar.dma_start(rms_b[:], _bcast_ap(rms_in[:], P))

            prep_state = None
            for _rep in range(reps):
                if mode == "mm_loop" and prep_state is not None:
                    emit_matmul(nc, aq_all_a, aq_all_b, out_ext,
                                prep_state[0], prep_state[1],
                                lhs_p, psum_p, out_p, _rep)
                    continue
                st = emit_body(nc, tc, rg, x_in, w_in, rms_in, out_ext,
                               stats_loc, stats_all, wstat_loc, wstat_all,
                               scal_dram, scbd_dram, aq_loc, aq_all_a, aq_all_b,
                               wres_p, bt_p, rms_p, xz_p, aq_p, st_p, lhs_p,
                               psum_p, out_p, small_p, eps_t, _rep, stop_after,
                               rms_b)
                if st is not None:
                    emit_matmul(nc, aq_all_a, aq_all_b, out_ext, st[0], st[1],
                                lhs_p, psum_p, out_p, _rep)
                    prep_state = st

    _split_waits(nc)
    return nc


def emit_body(nc, tc, rg, x_in, w_in, rms_in, out_ext,
              stats_loc, stats_all, wstat_loc, wstat_all,
              scal_dram, scbd_dram, aq_loc, aq_all_a, aq_all_b,
              wres_p, bt_p, rms_p, xz_p, aq_p, st_p, lhs_p,
              psum_p, out_p, small_p, eps_t, rep, stop_after=None, rms_b=None):
    if True:
        if True:

            # ---------- W shard: load resident + abs-sum stats ----------
            w_res = wres_p.tile([P, KT, N_LOC], F32)
            nc.scalar.dma_start(
                w_res[:], w_in[:, :].rearrange("(kt p) n -> p kt n", p=P)
            )
            wsum32 = small_p.tile([P, KT], F32)
            nc.vector.tensor_reduce(
                out=wsum32,
                in_=w_res[:],
                axis=AX.X,
                op=ALU.add,
                apply_absolute_value=True,
            )
            pp2 = small_p.tile([P, 2], F32)
            nc.vector.tensor_reduce(out=pp2[:, 1:2], in_=wsum32, axis=AX.X, op=ALU.add)

            # ---------- x rows: RMS norm + local absmax ----------
            amax_mt = small_p.tile([P, MT_LOC], F32)
            z_tiles = []
            r_tiles = []
            for mt in range(MT_LOC):
                xz = xz_p.tile([P, K], F32)
                nc.scalar.dma_start(xz[:], x_in[mt * P : (mt + 1) * P, :])
                xg = xz[:].rearrange("p (g d) -> p g d", d=512)
                stats6 = st_p.tile([P, 8, 6], F32)
                for g in range(8):
                    nc.vector.bn_stats(out=stats6[:, g, :], in_=xg[:, g, :])
                mv = st_p.tile([P, 2], F32)
                nc.vector.bn_aggr(out=mv, in_=stats6[:])
                # mean(x^2) = var + mean^2
                msq = st_p.tile([P, 1], F32)
                nc.vector.tensor_tensor(
                    out=msq, in0=mv[:, 0:1], in1=mv[:, 0:1], op=ALU.mult
                )
                nc.vector.tensor_tensor(out=msq, in0=msq, in1=mv[:, 1:2], op=ALU.add)
                # r = 1/sqrt(msq + eps)
                r_t = st_p.tile([P, 1], F32)
                nc.scalar.activation(
                    out=r_t,
                    in_=msq,
                    func=mybir.ActivationFunctionType.Sqrt,
                    bias=eps_t,
                    scale=1.0,
                )
                nc.vector.reciprocal(out=r_t, in_=r_t)
                # zz = x * rms_weight ; per-row absmax of zz
                amax_raw = st_p.tile([P, 1], F32)
                nc.vector.tensor_tensor(
                    out=xz[:], in0=xz[:], in1=rms_b[:], op=ALU.mult
                )
                nc.vector.tensor_reduce(
                    out=amax_raw,
                    in_=xz[:],
                    axis=AX.X,
                    op=ALU.max,
                    apply_absolute_value=True,
                )
                # row absmax of z = absmax(x*rms) * r  (r > 0)
                nc.vector.tensor_tensor(
                    out=amax_mt[:, mt : mt + 1], in0=amax_raw, in1=r_t, op=ALU.mult
                )
                r_tiles.append(r_t)
                z_tiles.append(xz)

            nc.vector.tensor_reduce(
                out=pp2[:, 0:1], in_=amax_mt[:], axis=AX.X, op=ALU.max
            )

            # ---------- AG1: merged stats partials ([P,2] per rank) ----------
            nc.scalar.dma_start(stats_loc[:].rearrange("(p t) -> p t", p=P), pp2[:])
            nc.gpsimd.collective_compute(
                "AllGather",
                ALU.bypass,
                replica_groups=rg,
                ins=[stats_loc[:]],
                outs=[stats_all[:]],
            )
            sball = small_p.tile([P, R * P * 2], F32)
            nc.scalar.dma_start(sball[:], _bcast_ap(stats_all[:], P))
            v = sball[:].rearrange("p (r t) -> p r t", t=2)
            gmax = small_p.tile([P, 1], F32)
            nc.vector.tensor_reduce(out=gmax, in_=v[:, :, 0:1], axis=AX.XY, op=ALU.max)
            nc.vector.tensor_scalar_max(out=gmax, in0=gmax, scalar1=Q_CLIP)
            a_s = small_p.tile([P, 1], F32)
            nc.vector.reciprocal(out=a_s, in_=gmax)
            nc.vector.tensor_scalar_mul(out=a_s, in0=a_s, scalar1=127.0)

            gsum = small_p.tile([P, 1], F32)
            nc.vector.tensor_reduce(out=gsum, in_=v[:, :, 1:2], axis=AX.XY, op=ALU.add)
            nc.vector.tensor_scalar(
                out=gsum,
                in0=gsum,
                scalar1=1.0 / (K * N),
                scalar2=Q_CLIP,
                op0=ALU.mult,
                op1=ALU.max,
            )
            b_s = small_p.tile([P, 1], F32)
            nc.vector.reciprocal(out=b_s, in_=gsum)
            dq_b = small_p.tile([P, 1], F32)
            nc.vector.tensor_tensor(out=dq_b, in0=gmax, in1=gsum, op=ALU.mult)
            nc.vector.tensor_scalar_mul(out=dq_b, in0=dq_b, scalar1=1.0 / 127.0)

            if stop_after == "scales":
                nc.scalar.dma_start(out_ext[0:P, 0:1], dq_b[:])
                return

            # ---------- activation quant (bf16 ints) + split allgather -------
            for mt in range(MT_LOC):
                z = z_tiles[mt]
                rs_c = st_p.tile([P, 1], F32, tag="rs_c", name=f"rs_{mt}")
                nc.vector.tensor_tensor(
                    out=rs_c, in0=r_tiles[mt], in1=a_s, op=ALU.mult
                )
                # z <- z * (r*a_scale) + MAGIC ; aq <- z - MAGIC (round nearest)
                nc.vector.tensor_scalar(
                    out=z[:],
                    in0=z[:],
                    scalar1=rs_c,
                    scalar2=MAGIC,
                    op0=ALU.mult,
                    op1=ALU.add,
                )
                aq_t = aq_p.tile([P, K], BF16)
                nc.vector.tensor_scalar(
                    out=aq_t[:], in0=z[:], scalar1=MAGIC, scalar2=None, op0=ALU.subtract
                )
                nc.scalar.dma_start(aq_loc[mt * P : (mt + 1) * P, :], aq_t[:])
                nc.gpsimd.collective_compute(
                    "AllGather",
                    ALU.bypass,
                    replica_groups=rg,
                    ins=[aq_loc[mt * P : (mt + 1) * P, :]],
                    outs=[(aq_all_a if mt == 0 else aq_all_b)[:, :]],
                )

            # ---------- weight quant: ternary bf16 ----------
            bt = bt_p.tile([P, KT, N_LOC], BF16)
            nc.vector.tensor_scalar(
                out=w_res[:],
                in0=w_res[:],
                scalar1=b_s[:, 0:1],
                scalar2=MAGIC,
                op0=ALU.mult,
                op1=ALU.add,
            )
            nc.vector.tensor_scalar(
                out=w_res[:],
                in0=w_res[:],
                scalar1=MAGIC,
                scalar2=1.0,
                op0=ALU.subtract,
                op1=ALU.min,
            )
            nc.vector.tensor_scalar(
                out=bt[:], in0=w_res[:], scalar1=-1.0, scalar2=None, op0=ALU.max
            )

            if stop_after == "quant":
                o_t = out_p.tile([P, N_LOC], F32)
                nc.vector.tensor_scalar_mul(out=o_t[:], in0=bt[:, 0, :], scalar1=1.0)
                nc.scalar.dma_start(out_ext[0:P, :], o_t[:])
                return None
            return (bt, dq_b)


def emit_matmul(nc, aq_all_a, aq_all_b, out_ext, bt, scal_b,
                lhs_p, psum_p, out_p, rep):
    if True:
        if True:
            # ---------- matmul: out[m, n_loc] = A_q @ B_t, dequant fused -----
            # half 0 consumes aq_all_a (each rank's first m-tile: global
            # m-tiles 0,2,4,...), half 1 consumes aq_all_b (1,3,5,...), so the
            # second allgather overlaps the first half's matmuls.
            HALF_MT = 8
            for half in range(2):
                psums = [
                    psum_p.tile([P, N_LOC], F32, tag="ps", name=f"ps_{half}_{i}")
                    for i in range(HALF_MT)
                ]
                aq_src = aq_all_a if half == 0 else aq_all_b
                for kt in range(KT):
                    lhsT = lhs_p.tile(
                        [P, HALF_MT * P], BF16, tag="lhsT", name=f"lh_{half}_{kt}"
                    )
                    nc.sync.dma_start_transpose(
                        lhsT[:], aq_src[:, kt * P : (kt + 1) * P]
                    )
                    for mt in range(HALF_MT):
                        nc.tensor.matmul(
                            psums[mt][:],
                            lhsT[:, mt * P : (mt + 1) * P],
                            bt[:, kt, :],
                            start=(kt == 0),
                            stop=(kt == KT - 1),
                        )
                for mt in range(HALF_MT):
                    o_t = out_p.tile([P, N_LOC], F32)
                    nc.vector.tensor_scalar_mul(
                        out=o_t[:], in0=psums[mt][:], scalar1=scal_b[:, 0:1]
                    )
                    gm = 2 * mt + half
                    nc.scalar.dma_start(out_ext[gm * P : (gm + 1) * P, :], o_t[:])


_CACHE = {}


def _get_nc():
    if "nc" not in _CACHE:
        _CACHE["nc"] = build_kernel()
    return _CACHE["nc"]


def make_in_maps(x, weight, rms_weight):
    x = np.ascontiguousarray(np.asarray(x, dtype=np.float32)).reshape(M, K)
    weight = np.asarray(weight, dtype=np.float32)
    rms_weight = np.ascontiguousarray(np.asarray(rms_weight, dtype=np.float32))
    return [
        {
            "x_loc": np.ascontiguousarray(x[c * M_LOC : (c + 1) * M_LOC]),
            "w_loc": np.ascontiguousarray(weight[:, c * N_LOC : (c + 1) * N_LOC]),
            "rms_w": rms_weight,
        }
        for c in range(R)
    ]


def assemble_out(results):
    out = np.concatenate([results[c]["out_loc"] for c in range(R)], axis=1)
    return out.reshape(1, M, N)


def kernel(x, weight, rms_weight):
    nc = _get_nc()
    in_maps = make_in_maps(x, weight, rms_weight)
    res = run_bass_kernel_spmd(nc, in_maps, core_ids=list(range(R)))
    return assemble_out(res.results)



# revision 2
# speedup vs baseline: 16.7104x; 16.7104x over previous
"""BitLinear v5 on 8 Trainium2 NeuronCores.

Key structure (v2 -> v3: int8 wire, 8 AG chunks, coarse strided DMAs,
engine rebalance):
  - w shard [4096, 512] f32 resident, 8 chunked loads (sync queue); |w|
    row-sums per chunk on DVE.
  - x rows [256, 4096] in 4 k-chunks per m-tile (scalar queue): ACT
    Square+accum_out mean-square, DVE z = x*rms and row absmax, pipelined
    with the DMAs.
  - TWO tiny AllGathers: amax partials early (a_scale unblocks activation
    quant), |w|-sum partials later (b_scale only gates ternarize).
  - Activation quant: ACT Identity+MAGIC (exact round-half-even), 8
    k-slices; each [128,128] block DMA-transposed (SBUF->SBUF) then DVE
    cast bf16->int8; ONE push DMA per slice; 8 int8 AllGathers overlap
    the matmul (int8 wire halves both link and HBM traffic).
  - Ternarize: ACT round + ACT Relu fold + POOL min/sub pass -> -B_t
    bf16 (dequant scalar negated).
  - Matmul: per (half, chunk): 4 strided 256KB DMAs land [128,KTJ,4,256]
    int8; per k-tile a DVE cast makes the bf16 lhsT; 8 m-tile PSUM
    accumulation over 32 k-tiles; ACT Copy dequant PSUM->SBUF; out DMA.
"""

import os
import sys

import numpy as np

for _p in ("/opt/trn_rl_repo", "/root/.axon_site/_ro/trn_rl_repo"):
    if os.path.isdir(_p) and _p not in sys.path:
        sys.path.append(_p)

import concourse.bass as bass
import concourse.tile as tile
from concourse import mybir
from concourse.bass_utils import run_bass_kernel_spmd

R = 8
M, K, N = 2048, 4096, 4096
M_LOC = M // R  # 256
N_LOC = N // R  # 512
P = 128
KT = K // P  # 32
MT_LOC = M_LOC // P  # 2
J = 8  # AG chunks / quant slices / tern chunks
KTJ = KT // J  # 4 k-tiles per chunk
XC = 4  # x k-chunks per m-tile
XW = K // XC  # 1024
EPS_RMS = 1e-6
Q_CLIP = 1e-5
MAGIC = 12582912.0
F32 = mybir.dt.float32
BF16 = mybir.dt.bfloat16
I8 = mybir.dt.int8
AX = mybir.AxisListType
ALU = mybir.AluOpType
AFT = mybir.ActivationFunctionType


def _split_waits(nc, max_waits=1):
    counter = 0
    for f in nc.m.functions:
        for blk in f.blocks:
            new_insts = []
            for inst in blk.instructions:
                si = getattr(inst, "sync_info", None)
                waits = list(si.on_wait) if si is not None and si.on_wait else []
                if len(waits) > max_waits:
                    excess = waits[: len(waits) - max_waits]
                    keep = waits[len(waits) - max_waits :]
                    for i in range(0, len(excess), max_waits):
                        counter += 1
                        nop = mybir.InstNoOp(
                            name=f"waitsplit_{counter}_{inst.name}", ins=[], outs=[]
                        )
                        nop.engine = inst.engine
                        nop.bass_nofuse = True
                        nop.sync_info = mybir.SyncInfo(
                            on_wait=list(excess[i : i + max_waits]), on_update=[]
                        )
                        new_insts.append(nop)
                    si.on_wait = keep
                    inst.sync_info = si
                new_insts.append(inst)
            blk.instructions[:] = new_insts


def _bcast_ap(ap, p):
    return bass.AP(tensor=ap.tensor, offset=ap.offset, ap=[[0, p]] + list(ap.ap))


def build_kernel(reps=1, stop_after=None, mode=None):
    nc = bass.Bass(num_devices=R)
    rg = [list(range(R))]

    x_in = nc.declare_dram_parameter("x_loc", [M_LOC, K], F32, isOutput=False)
    w_in = nc.declare_dram_parameter("w_loc", [K, N_LOC], F32, isOutput=False)
    rms_in = nc.declare_dram_parameter("rms_w", [K], F32, isOutput=False)
    out_ext = nc.declare_dram_parameter("out_loc", [M, N_LOC], F32, isOutput=True)

    amax_loc = nc.dram_tensor("amax_loc", [P], F32)
    amax_all = nc.dram_tensor("amax_all", [R * P], F32, addr_space="Shared")
    wsum_loc = nc.dram_tensor("wsum_loc", [P], F32)
    wsum_all = nc.dram_tensor("wsum_all", [R * P], F32, addr_space="Shared")
    aq_dram = nc.dram_tensor("aq_dram", [M_LOC, K], BF16)
    # two uneven k-chunks: A = k-tiles 0..7 (small, unblocks the matmul
    # early), B = k-tiles 8..31 (big, overlaps the matmul)
    KTA = 8
    KTB = KT - KTA
    aqt_loc = [
        nc.dram_tensor("aqt_locA", [KTA * P, M_LOC], I8),
        nc.dram_tensor("aqt_locB", [KTB * P, M_LOC], I8),
    ]
    aqt_all = [
        nc.dram_tensor("aqt_allA", [R * KTA * P, M_LOC], I8, addr_space="Shared"),
        nc.dram_tensor("aqt_allB", [R * KTB * P, M_LOC], I8, addr_space="Shared"),
    ]

    from contextlib import ExitStack

    with tile.TileContext(nc) as tc:
        ctxs = [
            tc.tile_pool(name="wres", bufs=1),
            tc.tile_pool(name="btres", bufs=1),
            tc.tile_pool(name="rmsp", bufs=1),
            tc.tile_pool(name="xz", bufs=2),
            tc.tile_pool(name="aq", bufs=2),
            tc.tile_pool(name="aqtb", bufs=4),
            tc.tile_pool(name="aqti", bufs=2),
            tc.tile_pool(name="st", bufs=4),
            tc.tile_pool(name="lhsi", bufs=2),
            tc.tile_pool(name="lhsb", bufs=4),
            tc.tile_pool(name="psum", bufs=8, space="PSUM"),
            tc.tile_pool(name="outp", bufs=4),
            tc.tile_pool(name="small", bufs=1),
        ]
        with ExitStack() as es:
            (wres_p, bt_p, rms_p, xz_p, aq_p, aqtb_p, aqti_p, st_p, lhsi_p,
             lhsb_p, psum_p, out_p, small_p) = [es.enter_context(c) for c in ctxs]

            eps_t = small_p.tile([P, 1], F32)
            nc.vector.memset(eps_t, EPS_RMS)
            b_mag = small_p.tile([P, 1], F32)
            nc.vector.memset(b_mag, MAGIC)
            b_nmag = small_p.tile([P, 1], F32)
            nc.vector.memset(b_nmag, -MAGIC)
            b_mag1 = small_p.tile([P, 1], F32)
            nc.vector.memset(b_mag1, MAGIC + 1.0)
            bias_t = (b_mag, b_nmag, b_mag1)
            rms_b = rms_p.tile([P, K], F32)
            nc.scalar.dma_start(rms_b[:], _bcast_ap(rms_in[:], P))

            prep_state = None
            for _rep in range(reps):
                if mode == "mm_loop" and prep_state is not None:
                    emit_matmul(nc, aqt_all, out_ext, prep_state[0], prep_state[1],
                                lhsi_p, lhsb_p, psum_p, out_p)
                    continue
                st = emit_body(nc, rg, x_in, w_in, out_ext,
                               amax_loc, amax_all, wsum_loc, wsum_all,
                               aq_dram, aqt_loc, aqt_all,
                               wres_p, bt_p, xz_p, aq_p, aqtb_p, aqti_p, st_p,
                               small_p, psum_p, eps_t, rms_b, bias_t, stop_after)
                if st is not None:
                    emit_matmul(nc, aqt_all, out_ext, st[0], st[1],
                                lhsi_p, lhsb_p, psum_p, out_p)
                    prep_state = st

    _split_waits(nc)
    return nc


def emit_body(nc, rg, x_in, w_in, out_ext, amax_loc, amax_all, wsum_loc, wsum_all,
              aq_dram, aqt_loc, aqt_all, wres_p, bt_p, xz_p, aq_p, aqtb_p, aqti_p, st_p,
              small_p, psum_p, eps_t, rms_b, bias_t, stop_after=None):
    b_mag, b_nmag, b_mag1 = bias_t

    # ---------- W: 8 chunked loads on sync queue ----------
    w_res = wres_p.tile([P, KT, N_LOC], F32)
    for c in range(J):
        k0 = c * KTJ
        nc.sync.dma_start(
            w_res[:, k0 : k0 + KTJ, :],
            w_in[k0 * P : (k0 + KTJ) * P, :].rearrange("(kt p) n -> p kt n", p=P),
        )

    # ---------- x: 4 k-chunks per m-tile, pipelined stats ----------
    z_tiles = []
    r_tiles = []
    NSQ = 2 * XC  # Square sub-chunks sized to a small reused scratch
    sq4 = [st_p.tile([P, NSQ], F32, tag="sq", name=f"sq_{mt}") for mt in range(MT_LOC)]
    am4 = [st_p.tile([P, XC], F32, tag="am", name=f"am_{mt}") for mt in range(MT_LOC)]
    sq_scr = small_p.tile([P, 512], F32)
    for mt in range(MT_LOC):
        z_tiles.append(xz_p.tile([P, K], F32, tag="xz", name=f"xz_{mt}"))
    for c in range(XC):
        sl = slice(c * XW, (c + 1) * XW)
        for mt in range(MT_LOC):
            z = z_tiles[mt]
            nc.scalar.dma_start(z[:, sl], x_in[mt * P : (mt + 1) * P, sl])
            # sum(x^2) on ACT (result only in accum_out; main out goes to a
            # small reused scratch — Square instrs serialize on ACT anyway)
            for h in range(2):
                hs = slice(c * XW + h * 512, c * XW + (h + 1) * 512)
                nc.scalar.activation(
                    out=sq_scr[:], in_=z[:, hs], func=AFT.Square,
                    accum_out=sq4[mt][:, 2 * c + h : 2 * c + h + 1],
                )
            # z = x * rms ; chunk absmax
            nc.vector.tensor_tensor(
                out=z[:, sl], in0=z[:, sl], in1=rms_b[:, sl], op=ALU.mult
            )
            nc.vector.tensor_reduce(
                out=am4[mt][:, c : c + 1], in_=z[:, sl], axis=AX.X, op=ALU.max,
                apply_absolute_value=True,
            )

    amax_mt = small_p.tile([P, MT_LOC], F32)
    for mt in range(MT_LOC):
        # r = 1/sqrt(sum(x^2)/K + eps)
        ssum = st_p.tile([P, 1], F32, tag="ssum", name=f"ss_{mt}")
        nc.vector.tensor_reduce(out=ssum, in_=sq4[mt][:], axis=AX.X, op=ALU.add)
        r_t = st_p.tile([P, 1], F32, tag="rt", name=f"r_{mt}")
        nc.scalar.activation(
            out=r_t, in_=ssum, func=AFT.Sqrt, bias=eps_t, scale=1.0 / K
        )
        nc.vector.reciprocal(out=r_t, in_=r_t)
        r_tiles.append(r_t)
        amr = st_p.tile([P, 1], F32, tag="amr", name=f"amr_{mt}")
        nc.vector.tensor_reduce(out=amr, in_=am4[mt][:], axis=AX.X, op=ALU.max)
        nc.vector.tensor_tensor(
            out=amax_mt[:, mt : mt + 1], in0=amr, in1=r_t, op=ALU.mult
        )

    pp_a = small_p.tile([P, 1], F32)
    nc.vector.tensor_reduce(out=pp_a, in_=amax_mt[:], axis=AX.X, op=ALU.max)
    nc.scalar.dma_start(amax_loc[:].rearrange("(p o) -> p o", p=P), pp_a[:])
    nc.gpsimd.collective_compute(
        "AllGather", ALU.bypass, replica_groups=rg,
        ins=[amax_loc[:]], outs=[amax_all[:]],
    )
    sball_a = small_p.tile([P, R * P], F32)
    nc.scalar.dma_start(sball_a[:], _bcast_ap(amax_all[:], P))
    gmax = small_p.tile([P, 1], F32)
    nc.vector.tensor_reduce(out=gmax, in_=sball_a[:], axis=AX.X, op=ALU.max)
    nc.vector.tensor_scalar_max(out=gmax, in0=gmax, scalar1=Q_CLIP)
    a_s = small_p.tile([P, 1], F32)
    nc.vector.reciprocal(out=a_s, in_=gmax)
    nc.vector.tensor_scalar_mul(out=a_s, in0=a_s, scalar1=127.0)
    rs_c = []
    for mt in range(MT_LOC):
        rs = st_p.tile([P, 1], F32, tag="rs", name=f"rs_{mt}")
        nc.vector.tensor_tensor(out=rs, in0=r_tiles[mt], in1=a_s, op=ALU.mult)
        rs_c.append(rs)

    # ---------- |w| sums (DVE, after the a-scale critical chain) ----------
    wsum_c = small_p.tile([P, J], F32)
    for c in range(J):
        k0 = c * KTJ
        nc.vector.tensor_reduce(
            out=wsum_c[:, c : c + 1], in_=w_res[:, k0 : k0 + KTJ, :],
            axis=AX.XY, op=ALU.add, apply_absolute_value=True,
        )
    pp_b = small_p.tile([P, 1], F32)
    nc.vector.tensor_reduce(out=pp_b, in_=wsum_c, axis=AX.X, op=ALU.add)
    nc.scalar.dma_start(wsum_loc[:].rearrange("(p o) -> p o", p=P), pp_b[:])

    # ---------- quant slices + transposes + int8 push + AG chunks ----------
    bt = bt_p.tile([P, KT, N_LOC], BF16)
    aq_tiles = [aq_p.tile([P, K], BF16, tag="aq", name=f"aq_{mt}")
                for mt in range(MT_LOC)]
    SL = K // J  # 512 k-cols per slice

    b_s = small_p.tile([P, 1], F32)
    gsum = small_p.tile([P, 1], F32)
    dq_b = small_p.tile([P, 1], F32)
    sball_b = small_p.tile([P, R * P], F32)
    wsum_ag_done = False

    for j in range(J):
        sl = slice(j * SL, (j + 1) * SL)
        for mt in range(MT_LOC):
            z = z_tiles[mt]
            nc.scalar.activation(
                out=z[:, sl], in_=z[:, sl], func=AFT.Identity,
                bias=b_mag, scale=rs_c[mt][:, 0:1],
            )
            nc.scalar.activation(
                out=aq_tiles[mt][:, sl], in_=z[:, sl], func=AFT.Identity,
                bias=b_nmag, scale=1.0,
            )
        for mt in range(MT_LOC):
            nc.scalar.dma_start(
                aq_dram[mt * P : (mt + 1) * P, sl], aq_tiles[mt][:, sl]
            )
        aqt_i8 = aqti_p.tile([P, KTJ, M_LOC], I8, tag="aqti", name=f"aqti_{j}")
        for kk in range(KTJ):
            kt = j * KTJ + kk
            aqt_bf = aqtb_p.tile([P, M_LOC], BF16, tag="aqtb", name=f"aqtb_{kt}")
            nc.scalar.dma_start_transpose(
                aqt_bf[:], aq_dram[:, kt * P : (kt + 1) * P]
            )
            nc.vector.tensor_scalar_mul(
                out=aqt_i8[:, kk, :], in0=aqt_bf[:], scalar1=1.0
            )
        # push this slice into its chunk tensor (A: j<2, B: j>=2)
        if j < 2:
            dst = aqt_loc[0][j * KTJ * P : (j + 1) * KTJ * P, :]
        else:
            dst = aqt_loc[1][(j - 2) * KTJ * P : (j - 1) * KTJ * P, :]
        nc.scalar.dma_start(dst.rearrange("(kt p) m -> p kt m", p=P), aqt_i8[:])
        if j == 1:
            nc.gpsimd.collective_compute(
                "AllGather", ALU.bypass, replica_groups=rg,
                ins=[aqt_loc[0][:]], outs=[aqt_all[0][:]],
            )
            # wsum AG rides between the two aq chunk AGs
            nc.gpsimd.collective_compute(
                "AllGather", ALU.bypass, replica_groups=rg,
                ins=[wsum_loc[:]], outs=[wsum_all[:]],
            )
            nc.scalar.dma_start(sball_b[:], _bcast_ap(wsum_all[:], P))
            nc.vector.tensor_reduce(
                out=gsum, in_=sball_b[:], axis=AX.X, op=ALU.add
            )
            nc.vector.tensor_scalar(
                out=gsum, in0=gsum, scalar1=1.0 / (K * N), scalar2=Q_CLIP,
                op0=ALU.mult, op1=ALU.max,
            )
            nc.vector.reciprocal(out=b_s, in_=gsum)
            nc.vector.tensor_tensor(out=dq_b, in0=gmax, in1=gsum, op=ALU.mult)
            nc.vector.tensor_scalar_mul(out=dq_b, in0=dq_b, scalar1=-1.0 / 127.0)
        elif j == J - 1:
            nc.gpsimd.collective_compute(
                "AllGather", ALU.bypass, replica_groups=rg,
                ins=[aqt_loc[1][:]], outs=[aqt_all[1][:]],
            )

    # ternarize (after b_s): -B_t = min(relu(1 - round(w*b_s)), 2) - 1
    for j in range(J):
        k0 = j * KTJ
        wsl = w_res[:, k0 : k0 + KTJ, :]
        nc.scalar.activation(
            out=wsl, in_=wsl, func=AFT.Identity, bias=b_mag, scale=b_s[:, 0:1]
        )
        nc.scalar.activation(
            out=wsl, in_=wsl, func=AFT.Relu, bias=b_mag1, scale=-1.0
        )
        nc.vector.tensor_scalar(
            out=bt[:, k0 : k0 + KTJ, :], in0=wsl, scalar1=2.0, scalar2=1.0,
            op0=ALU.min, op1=ALU.subtract,
        )

    if stop_after == "quant":
        o_t = st_p.tile([P, N_LOC], F32, tag="qout")
        nc.vector.tensor_scalar_mul(out=o_t[:], in0=bt[:, 0, :], scalar1=1.0)
        nc.scalar.dma_start(out_ext[0:P, :], o_t[:])
        return None
    return (bt, dq_b)


def emit_matmul(nc, aqt_all, out_ext, bt, dq_b, lhsi_p, lhsb_p, psum_p, out_p):
    HALF_MT = 8
    for half in range(2):
        psums = [
            psum_p.tile([P, N_LOC], F32, tag="ps", name=f"ps_{half}_{i}")
            for i in range(HALF_MT)
        ]
        KTA = 8
        for j in range(J):
            lhs_i8 = lhsi_p.tile([P, KTJ, 4, M_LOC], I8, tag="lhsi",
                                 name=f"li_{half}_{j}")
            if j < 2:
                src, ktc, goff = aqt_all[0], KTA, j * KTJ * P
            else:
                src, ktc, goff = aqt_all[1], KT - KTA, (j - 2) * KTJ * P
            for i in range(4):
                r0 = (half * 4 + i) * ktc * P + goff
                nc.sync.dma_start(
                    lhs_i8[:, :, i, :],
                    src[r0 : r0 + KTJ * P, :].rearrange(
                        "(kt p) m -> p kt m", p=P
                    ),
                )
            for kk in range(KTJ):
                kt = j * KTJ + kk
                lhsT = lhsb_p.tile([P, HALF_MT * P], BF16, tag="lhsT",
                                   name=f"lh_{half}_{kt}")
                nc.vector.tensor_scalar_mul(
                    out=lhsT[:],
                    in0=lhs_i8[:, kk, :, :].rearrange("p i m -> p (i m)"),
                    scalar1=1.0,
                )
                for mt in range(HALF_MT):
                    nc.tensor.matmul(
                        psums[mt][:],
                        lhsT[:, mt * P : (mt + 1) * P],
                        bt[:, kt, :],
                        start=(kt == 0),
                        stop=(kt == KT - 1),
                    )
        for mt in range(HALF_MT):
            o_t = out_p.tile([P, N_LOC], F32)
            nc.scalar.activation(
                out=o_t[:], in_=psums[mt][:], func=AFT.Copy, scale=dq_b[:, 0:1]
            )
            gm = half * HALF_MT + mt
            nc.scalar.dma_start(out_ext[gm * P : (gm + 1) * P, :], o_t[:])


_CACHE = {}


def _get_nc():
    if "nc" not in _CACHE:
        _CACHE["nc"] = build_kernel()
    return _CACHE["nc"]


def make_in_maps(x, weight, rms_weight):
    x = np.ascontiguousarray(np.asarray(x, dtype=np.float32)).reshape(M, K)
    weight = np.asarray(weight, dtype=np.float32)
    rms_weight = np.ascontiguousarray(np.asarray(rms_weight, dtype=np.float32))
    return [
        {
            "x_loc": np.ascontiguousarray(x[c * M_LOC : (c + 1) * M_LOC]),
            "w_loc": np.ascontiguousarray(weight[:, c * N_LOC : (c + 1) * N_LOC]),
            "rms_w": rms_weight,
        }
        for c in range(R)
    ]


def assemble_out(results):
    out = np.concatenate([results[c]["out_loc"] for c in range(R)], axis=1)
    return out.reshape(1, M, N)


def kernel(x, weight, rms_weight):
    nc = _get_nc()
    in_maps = make_in_maps(x, weight, rms_weight)
    res = run_bass_kernel_spmd(nc, in_maps, core_ids=list(range(R)))
    return assemble_out(res.results)
